# revision 1
# baseline (speedup 1.0000x reference)
"""ArcFace loss on 8 TRN2 NeuronCores (Bass/Tile, class-sharded classifier).

Math: since cos(arccos(clip(x))) == clip(x), non-target logits are just
SCALE*clip(cosine). Only the target-class logit needs the margin:
l' = SCALE*(x*cos(m) - sqrt(1-x^2)*sin(m)).  Logits are <= SCALE, so
logsumexp uses the fixed max SCALE=64: lse = 64 + log(sum exp(l-64)).
Each core owns C/8 = 3750 classes (padded to 3840 with zero rows),
computes partial sum_c exp(64*clip(cos)-64) for all 4096 rows, plus the
target-row dot/norm for labels it owns; one 48KB AllReduce combines
[S, t, q] and every core finishes the (tiny) scalar loss identically.
"""

import math
import os
import sys
import types

import numpy as np

import concourse.bass as bass
import concourse.mybir as mybir
import concourse.tile as tile
from concourse import bacc
from concourse.bass_utils import run_bass_kernel_spmd
from concourse.masks import make_identity


def _install_profile_hook():
    """Make BASS_TRACE=1 work under axon images whose antenv lacks
    axon_hooks: register a sys.modules shim + the ctypes NTFF hook."""
    try:
        import antenv.axon_hooks  # noqa: F401
        return
    except ImportError:
        pass
    holder = {"fn": None}
    mod = types.ModuleType("antenv.axon_hooks")
    mod.set_axon_ntff_profile_hook = lambda fn: holder.__setitem__("fn", fn)
    mod.get_axon_ntff_profile_hook = lambda: holder["fn"]
    sys.modules["antenv.axon_hooks"] = mod
    try:
        import antenv
        antenv.axon_hooks = mod
    except ImportError:
        pass
    try:
        from trn_agent_boot.trn_boot import _ntff_profile_via_ctypes
        so = "/opt/axon/libaxon_pjrt.so"
        if os.path.exists(so):
            mod.set_axon_ntff_profile_hook(_ntff_profile_via_ctypes(so))
    except Exception:
        pass


_install_profile_hook()

F32 = mybir.dt.float32
BF16 = mybir.dt.bfloat16
I32 = mybir.dt.int32

N, D, C = 4096, 512, 30000
NCORES = 8
CREAL = C // NCORES          # 3750 real classes per core
CS = 4096                    # padded shard rows (32 * 128, symmetric with N)
DUMMY = CREAL                # a guaranteed-zero row in every shard
NB = N // 128                # 32 n-blocks
CB = CS // 128               # 32 c-blocks
DCH = D // 128               # 4 contraction chunks
CCH = 512                    # matmul chunk (one psum bank of f32)
NCC = CS // CCH              # 8 c-chunks
NGRP = 4                     # transpose / availability groups
GR = NB // NGRP              # 8 row-blocks per group
SCALE = 64.0
MARGIN = 0.5
COS_M = math.cos(MARGIN)
SIN_M = math.sin(MARGIN)
HI = 1.0 - 1e-7              # upper cosine clip (reference semantics)
LO_BULK = -0.35              # lower clip: exp(64*-0.35-64)=e^-86.4 ~ 1e-38 ~ 0
LO_TGT = -1.0 + 1e-7         # exact lower clip for the target-class formula

AX = mybir.AluOpType
AF = mybir.ActivationFunctionType

LAST_RESULT = None           # test.py reads exec_time_ns from here


def _build():
    nc = bacc.Bacc("TRN2", target_bir_lowering=False, debug=False,
                   num_devices=NCORES)

    emb = nc.dram_tensor("emb", [N, D], F32, kind="ExternalInput")
    wsh = nc.dram_tensor("wsh", [CS, D], F32, kind="ExternalInput")
    lab = nc.dram_tensor("lab", [128, NB], I32, kind="ExternalInput")
    out = nc.dram_tensor("out", [1, 1], F32, kind="ExternalOutput")

    with tile.TileContext(nc) as tc:
        with (
            tc.tile_pool(name="pers", bufs=1) as pers,
            tc.tile_pool(name="strm", bufs=4) as strm,
            tc.tile_pool(name="evac", bufs=4) as evac,
            tc.tile_pool(name="ppmm", bufs=3, space="PSUM") as ppmm,
            tc.tile_pool(name="ppfin", bufs=1, space="PSUM") as ppfin,
            tc.tile_pool(name="dram", bufs=1, space="DRAM") as dram,
        ):
            # ---- constants / persistent state ----
            ones_col = pers.tile([128, 1], F32)
            nc.vector.memset(ones_col[:], 1.0)
            bias_m64 = pers.tile([128, 1], F32)
            nc.vector.memset(bias_m64[:], -SCALE)
            bias_p64 = pers.tile([128, 1], F32)
            nc.vector.memset(bias_p64[:], SCALE)

            lab_sb = pers.tile([128, NB], I32)
            nc.sync.dma_start(lab_sb[:], lab[:])

            # bf16 staging in DRAM, d-major so XBAR transpose reads are
            # contiguous
            e_stage = dram.tile([DCH, N, 128], BF16)
            w_stage = dram.tile([DCH, CS, 128], BF16)
            eT = [pers.tile([128, N], BF16, name=f"eT{d}") for d in range(DCH)]
            wT = [pers.tile([128, CS], BF16, name=f"wT{d}") for d in range(DCH)]

            nrm2 = pers.tile([128, CB], F32)
            nrm = pers.tile([128, CB], F32)
            rn = pers.tile([128, CB], F32)
            t_sb = pers.tile([128, NB], F32)
            q_sb = pers.tile([128, NB], F32)
            scols = pers.tile([128, NB * NGRP], F32)

            # deferred DVE reductions (sprinkled through the main loop so
            # they never head-of-line-block the evac work)
            pending = []
            wnat_tiles = {}
            cc1_state = {"done": False}

            def emit_pending(n_ops):
                for _ in range(min(n_ops, len(pending))):
                    pending.pop(0)()

            def main_unit(cg, nb):
                tlo = cg * 2 * CCH
                ps_cos = ppmm.tile([128, 2 * CCH], F32, name="ps_cos")
                for h in range(2):
                    for d in range(DCH):
                        nc.tensor.matmul(
                            ps_cos[:, h * CCH:(h + 1) * CCH],
                            lhsT=eT[d][:, nb * 128:(nb + 1) * 128],
                            rhs=wT[d][:, tlo + h * CCH:tlo + (h + 1) * CCH],
                            start=(d == 0), stop=(d == DCH - 1))
                exp_t = evac.tile([128, 2 * CCH], BF16, name="exp_t")
                nc.scalar.activation(exp_t[:], ps_cos[:], AF.Exp,
                                     bias=bias_m64[:], scale=SCALE)
                mn_t = evac.tile([128, 2 * CCH], BF16, name="mn_t")
                col = nb * NGRP + cg
                nc.vector.tensor_scalar(
                    out=mn_t[:], in0=exp_t[:], scalar1=1.0, scalar2=None,
                    op0=AX.min, op1=AX.add,
                    accum_out=scols[:, col:col + 1])

            for g in range(NGRP):
                ks = range(g * GR, (g + 1) * GR)
                for k in ks:
                    # W: load + row-norm^2 (ACT square+accum, one table)
                    w_nat = strm.tile([128, D], F32, name="w_nat", bufs=11)
                    wnat_tiles[k] = w_nat
                    nc.sync.dma_start(w_nat[:], wsh[k * 128:(k + 1) * 128, :])
                    wsq = strm.tile([128, D], F32, name="wsq", bufs=2)
                    nc.scalar.activation(wsq[:], w_nat[:], AF.Square,
                                         accum_out=nrm2[:, k:k + 1])
                    # E: load, gather, elementwise products on gpsimd,
                    # bf16 cast on DVE, d-major stage via sync
                    e_nat = strm.tile([128, D], F32, name="e_nat")
                    nc.sync.dma_start(e_nat[:], emb[k * 128:(k + 1) * 128, :])
                    wy = strm.tile([128, D], F32, name="wy")
                    nc.gpsimd.indirect_dma_start(
                        out=wy[:], out_offset=None, in_=wsh[:],
                        in_offset=bass.IndirectOffsetOnAxis(
                            ap=lab_sb[:, k:k + 1], axis=0),
                    )
                    ew = strm.tile([128, D], F32, name="ew", bufs=10)
                    nc.gpsimd.tensor_mul(ew[:], e_nat[:], wy[:])
                    wy2 = strm.tile([128, D], F32, name="wy2", bufs=10)
                    nc.gpsimd.tensor_mul(wy2[:], wy[:], wy[:])

                    def _red(kk=k, a=ew, b=wy2):
                        nc.vector.reduce_sum(t_sb[:, kk:kk + 1], a[:],
                                             axis=mybir.AxisListType.X)
                        nc.vector.reduce_sum(q_sb[:, kk:kk + 1], b[:],
                                             axis=mybir.AxisListType.X)
                    pending.append(_red)

                    e_bf = strm.tile([128, D], BF16, name="e_bf")
                    nc.vector.tensor_copy(e_bf[:], e_nat[:])
                    for d in range(DCH):
                        nc.sync.dma_start(
                            e_stage[d, k * 128:(k + 1) * 128, :],
                            e_bf[:, d * 128:(d + 1) * 128])
                # batched 1/||w|| for the group (one Sqrt table load)
                gs = slice(g * GR, (g + 1) * GR)
                nc.scalar.sqrt(nrm[:, gs], nrm2[:, gs])
                nc.vector.tensor_scalar_max(nrm[:, gs], nrm[:, gs], 1e-12)
                nc.vector.reciprocal(rn[:, gs], nrm[:, gs])
                # normalize + store (scalar queue: copy then its own store)
                for k in ks:
                    w_nrm = strm.tile([128, D], BF16, name="w_nrm")
                    nc.scalar.activation(w_nrm[:], wnat_tiles[k][:], AF.Copy,
                                         scale=rn[:, k:k + 1])
                    for d in range(DCH):
                        nc.scalar.dma_start(
                            w_stage[d, k * 128:(k + 1) * 128, :],
                            w_nrm[:, d * 128:(d + 1) * 128])
                # XBAR transposes for this group (sync queue)
                lo, hi = g * GR * 128, (g + 1) * GR * 128
                for d in range(DCH):
                    nc.sync.dma_start_transpose(
                        eT[d][:, lo:hi], e_stage[d, lo:hi, :])
                    nc.sync.dma_start_transpose(
                        wT[d][:, lo:hi], w_stage[d, lo:hi, :])
                # main blocks that just became feasible: cg == g, all ng<=g
                # plus earlier cgs' ng == g blocks
                for cg, ng in [(g, n2) for n2 in range(g + 1)] + \
                              [(c2, g) for c2 in range(g)]:
                    for nb in range(ng * GR, (ng + 1) * GR):
                        main_unit(cg, nb)
                        emit_pending(1 if len(pending) < 12 else 2)
                    if (g == NGRP - 1 and not pending
                            and not cc1_state["done"]):
                        cc1_state["done"] = True
                        cc1_in = dram.tile([2, 128, NB], F32)
                        cc1_out = dram.tile([2, 128, NB], F32)
                        nc.sync.dma_start(cc1_in[0], t_sb[:])
                        nc.sync.dma_start(cc1_in[1], q_sb[:])
                        nc.gpsimd.collective_compute(
                            "AllReduce", AX.add,
                            replica_groups=[list(range(NCORES))],
                            ins=[cc1_in[:]], outs=[cc1_out[:]])
                        t_tot = pers.tile([128, NB], F32)
                        q_tot = pers.tile([128, NB], F32)
                        nc.sync.dma_start(t_tot[:], cc1_out[0])
                        nc.sync.dma_start(q_tot[:], cc1_out[1])
            assert cc1_state["done"] and not pending

            # ---- collective #2: softmax partial sums ----
            s_n = pers.tile([128, NB], F32)
            nc.vector.reduce_sum(
                s_n[:],
                scols[:].rearrange("p (nb c) -> p nb c", c=NGRP),
                axis=mybir.AxisListType.X)
            cc2_in = dram.tile([128, NB], F32)
            cc2_out = dram.tile([128, NB], F32)
            nc.sync.dma_start(cc2_in[:], s_n[:])
            nc.gpsimd.collective_compute(
                "AllReduce", AX.add,
                replica_groups=[list(range(NCORES))],
                ins=[cc2_in[:]], outs=[cc2_out[:]])
            s_tot = pers.tile([128, NB], F32)
            nc.sync.dma_start(s_tot[:], cc2_out[:])

            # ---- final scalar loss (tail; replicated on every core) ----
            ny = pers.tile([128, NB], F32)
            nc.scalar.sqrt(ny[:], q_tot[:])
            nc.vector.tensor_scalar_max(ny[:], ny[:], 1e-12)
            rny = pers.tile([128, NB], F32)
            nc.vector.reciprocal(rny[:], ny[:])
            x = pers.tile([128, NB], F32)
            nc.vector.tensor_mul(x[:], t_tot[:], rny[:])       # cos_y
            nc.vector.tensor_scalar(out=x[:], in0=x[:], scalar1=HI,
                                    scalar2=LO_TGT, op0=AX.min, op1=AX.max)
            xsq = pers.tile([128, NB], F32)
            nc.scalar.square(xsq[:], x[:])
            s1mx = pers.tile([128, NB], F32)                   # sqrt(1-x^2)
            nc.scalar.activation(s1mx[:], xsq[:], AF.Sqrt, bias=1.0,
                                 scale=-1.0)
            lm = pers.tile([128, NB], F32)                     # margin logit
            nc.vector.tensor_scalar_mul(lm[:], x[:], SCALE * COS_M)
            sb_t = pers.tile([128, NB], F32)
            nc.vector.tensor_scalar_mul(sb_t[:], s1mx[:], SCALE * SIN_M)
            nc.vector.tensor_sub(lm[:], lm[:], sb_t[:])
            sub_t = pers.tile([128, NB], F32)
            nc.scalar.activation(sub_t[:], x[:], AF.Exp, bias=bias_m64[:],
                                 scale=SCALE)
            add_t = pers.tile([128, NB], F32)
            nc.scalar.activation(add_t[:], lm[:], AF.Exp, bias=bias_m64[:],
                                 scale=1.0)
            sadj = pers.tile([128, NB], F32)
            nc.vector.tensor_sub(sadj[:], s_tot[:], sub_t[:])
            nc.vector.tensor_add(sadj[:], sadj[:], add_t[:])
            lse = pers.tile([128, NB], F32)
            nc.scalar.activation(lse[:], sadj[:], AF.Ln)
            lossn = pers.tile([128, NB], F32)                  # loss - 64
            nc.vector.tensor_sub(lossn[:], lse[:], lm[:])
            red1 = pers.tile([128, 1], F32)
            nc.vector.reduce_sum(red1[:], lossn[:], axis=mybir.AxisListType.X)
            ps_fin = ppfin.tile([1, 1], F32, name="ps_fin")
            nc.tensor.matmul(ps_fin[:], lhsT=ones_col[:], rhs=red1[:],
                             start=True, stop=True)
            out_sb = pers.tile([1, 1], F32)
            nc.scalar.activation(out_sb[:], ps_fin[:], AF.Identity,
                                 bias=bias_p64[:1, :1], scale=1.0 / N)
            nc.sync.dma_start(out[:1, :1], out_sb[:])

    nc.finalize()
    return nc


_NC = None


def _get_nc():
    global _NC
    if _NC is None:
        _NC = _build()
    return _NC


def kernel(embeddings, labels, classifier_weights):
    global LAST_RESULT
    E = np.ascontiguousarray(np.asarray(embeddings, dtype=np.float32))
    W = np.ascontiguousarray(np.asarray(classifier_weights, dtype=np.float32))
    lab = np.asarray(labels).astype(np.int64).ravel()
    assert E.shape == (N, D) and W.shape == (C, D) and lab.shape == (N,)

    nc = _get_nc()
    in_maps = []
    for i in range(NCORES):
        lo, hi = i * CREAL, (i + 1) * CREAL
        wsh_i = np.zeros((CS, D), dtype=np.float32)
        wsh_i[:CREAL] = W[lo:hi]
        loc = np.where((lab >= lo) & (lab < hi), lab - lo, DUMMY)
        lab_i = np.ascontiguousarray(
            loc.reshape(NB, 128).T.astype(np.int32))  # [128, NB], n = nb*128+p
        in_maps.append({"emb": E, "wsh": wsh_i, "lab": lab_i})

    res = run_bass_kernel_spmd(nc, in_maps, core_ids=list(range(NCORES)))
    LAST_RESULT = res
    val = np.float32(res.results[0]["out"].reshape(())[()])
    return np.asarray(val, dtype=np.float32).reshape(())



# revision 2
# speedup vs baseline: 1.5599x; 1.5599x over previous
"""ArcFace loss on 8 TRN2 NeuronCores — fp8 DoubleRow rewrite.

vs baseline: host supplies E/W pre-transposed (pure layout), so no on-device
staging/transpose round trip; classifier matmul runs in fp8e4 with
perf_mode=DoubleRow (256-deep contraction, 2 fp8/cell); per-class 1/||w||
is broadcast across partitions with a rank-1 ones-outer-product matmul and
fused into the fp8 weight cast; q=||w_y||^2 and t=e.w_y come from fused
DVE tensor_tensor_reduce ops on gathered rows.  Math identical to baseline:
S = sum_c min(exp(64 cos - 64), 1), margin applied analytically to the
target logit, one AllReduce for [t,q], one for S.
"""

import math
import os
import sys
import types

import numpy as np

import concourse.bass as bass
import concourse.mybir as mybir
import concourse.tile as tile
from concourse import bacc
from concourse.bass_utils import run_bass_kernel_spmd


def _install_profile_hook():
    try:
        import antenv.axon_hooks  # noqa: F401
        return
    except ImportError:
        pass
    holder = {"fn": None}
    mod = types.ModuleType("antenv.axon_hooks")
    mod.set_axon_ntff_profile_hook = lambda fn: holder.__setitem__("fn", fn)
    mod.get_axon_ntff_profile_hook = lambda: holder["fn"]
    sys.modules["antenv.axon_hooks"] = mod
    try:
        import antenv
        antenv.axon_hooks = mod
    except ImportError:
        pass
    try:
        from trn_agent_boot.trn_boot import _ntff_profile_via_ctypes
        so = "/opt/axon/libaxon_pjrt.so"
        if os.path.exists(so):
            mod.set_axon_ntff_profile_hook(_ntff_profile_via_ctypes(so))
    except Exception:
        pass


_install_profile_hook()

F32 = mybir.dt.float32
BF16 = mybir.dt.bfloat16
F8 = mybir.dt.float8e4
I32 = mybir.dt.int32

N, D, C = 4096, 512, 30000
NCORES = int(os.environ.get("ARCFACE_NCORES", "8"))
CREAL = C // 8               # 3750 real classes per core
CSP = 3760                   # padded shard classes (multiple of 16)
DUMMY = CREAL                # zero row in wnat for non-owned labels
NB = N // 128                # 32 n-blocks
SE = 4.0                     # fp8 scale for embeddings
SW = 32.0                    # fp8 scale for normalized weights
SCALE = 64.0
MARGIN = 0.5
COS_M = math.cos(MARGIN)
SIN_M = math.sin(MARGIN)
HI = 1.0 - 1e-7
LO_TGT = -1.0 + 1e-7
DR = mybir.MatmulPerfMode.DoubleRow

# per-nb PSUM halves: (c0, width); widths chunked into <=512 matmuls
HALves = [(0, 2048), (2048, 1712)]
WCH = [(j * 512, 512) for j in range(7)] + [(3584, 176)]  # W prep chunks

AX = mybir.AluOpType
AF = mybir.ActivationFunctionType

LAST_RESULT = None
NO_TTR = os.environ.get("ARCFACE_NO_TTR", "0") == "1"
NO_RAF = os.environ.get("ARCFACE_NO_RAF", "0") == "1"
NO_TPATH = os.environ.get("ARCFACE_NO_TPATH", "0") == "1"


def _mm_chunks(cw):
    out = []
    off = 0
    while off < cw:
        sz = min(512, cw - off)
        out.append((off, sz))
        off += sz
    return out


def _build():
    nc = bacc.Bacc("TRN2", target_bir_lowering=False, debug=False,
                   num_devices=NCORES)

    eTd = nc.dram_tensor("eT", [D, N], F32, kind="ExternalInput")
    emb = nc.dram_tensor("emb", [N, D], F32, kind="ExternalInput")
    wTd = nc.dram_tensor("wT", [D, CSP], F32, kind="ExternalInput")
    wnat = nc.dram_tensor("wnat", [CREAL + 1, D], F32, kind="ExternalInput")
    lab = nc.dram_tensor("lab", [128, NB], I32, kind="ExternalInput")
    out = nc.dram_tensor("out", [1, 1], F32, kind="ExternalOutput")

    with tile.TileContext(nc) as tc:
        with (
            tc.tile_pool(name="pers", bufs=1) as pers,
            tc.tile_pool(name="wtf", bufs=1) as wtf,
            tc.tile_pool(name="rowp", bufs=2) as rowp,
            tc.tile_pool(name="etf", bufs=1) as etf,
            tc.tile_pool(name="wsqp", bufs=2) as wsqp,
            tc.tile_pool(name="evac", bufs=3) as evac,
            tc.tile_pool(name="mns", bufs=1) as mns,
            tc.tile_pool(name="enat", bufs=4) as enat,
            tc.tile_pool(name="wyp", bufs=4) as wyp,
            tc.tile_pool(name="tts", bufs=2) as tts,
            tc.tile_pool(name="pp", bufs=2, space="PSUM") as pp,
            tc.tile_pool(name="dram", bufs=1, space="DRAM") as dram,
        ):
            # ---- constants / persistent ----
            ones_row = pers.tile([1, 128], BF16)
            nc.vector.memset(ones_row[:], 1.0)
            ones_col = pers.tile([128, 1], F32)
            nc.vector.memset(ones_col[:], 1.0)
            ones_col_bf = pers.tile([128, 1], BF16)
            nc.vector.memset(ones_col_bf[:], 1.0)
            bias_m64 = pers.tile([128, 1], F32)
            nc.vector.memset(bias_m64[:], -SCALE)

            lab_sb = pers.tile([128, NB], I32)
            nc.sync.dma_start(lab_sb[:], lab[:])

            eT8 = pers.tile([128, 4, N], F8)
            wT8 = pers.tile([128, 4, CSP], F8)
            nrow = rowp.tile([1, CSP], F32, name="row")
            rn_bf = pers.tile([1, CSP], BF16)
            scols = pers.tile([128, NB * 2], F32)
            t_sb = pers.tile([128, NB], F32)
            q_sb = pers.tile([128, NB], F32)

            # ---- W load (one big tile) + per-chunk norms ----
            wTf = wtf.tile([128, 4, CSP], F32)
            for d in range(4):
                nc.sync.dma_start(wTf[:, d, :], wTd[d * 128:(d + 1) * 128, :])

            # squares -> bf16; ones-matmul partition-reduce into psum rows
            ps_n = {}
            for jj, (c0, cw) in enumerate(WCH):
                if jj % 4 == 0:
                    ps_n[jj // 4] = pp.tile([128, 2048], F32, name="ps")
                psn = ps_n[jj // 4]
                wsq = wsqp.tile([128, 4, 512], BF16, name="wsq")
                nc.scalar.activation(wsq[:, :, :cw], wTf[:, :, c0:c0 + cw],
                                     AF.Square)
                q0 = (jj % 4) * 512
                for d in range(4):
                    nc.tensor.matmul(psn[:1, q0:q0 + cw],
                                     lhsT=ones_col_bf[:],
                                     rhs=wsq[:, d, :cw],
                                     start=(d == 0), stop=(d == 3))
                nc.vector.tensor_copy(nrow[:, c0:c0 + cw],
                                      psn[:1, q0:q0 + cw])

            # ---- E load + fp8 cast (interleaves with W norms above via
            # queue scheduling; emitted here, runs on ACT) ----
            for d in range(4):
                eTf = etf.tile([128, N], F32, name="eTf")
                nc.sync.dma_start(eTf[:], eTd[d * 128:(d + 1) * 128, :])
                nc.scalar.activation(eT8[:, d, :], eTf[:], AF.Copy, scale=SE)

            # ---- rn row: SW/max(||w||,eps) on one partition ----
            srow = rowp.tile([1, CSP], F32, name="row")
            eps24 = pers.tile([1, 1], F32)
            nc.vector.memset(eps24[:], 1e-24)
            nc.scalar.activation(srow[:], nrow[:], AF.Sqrt,
                                 scale=1.0 / (SW * SW), bias=eps24[:])
            rrow = rowp.tile([1, CSP], F32, name="row")
            if NO_RAF:
                nc.vector.reciprocal(rrow[:], srow[:])
            else:
                nc.vector.reciprocal_approx_fast(rrow[:], srow[:])
            nc.vector.tensor_copy(rn_bf[:], rrow[:])

            # ---- broadcast rn across partitions via rank-1 matmul, then
            # fused normalize+fp8 cast of W ----
            rnb = {}
            for h, (c0, cw) in enumerate(HALves):
                rnb[h] = pp.tile([128, 2048], F32, name="ps")
                for off, sz in _mm_chunks(cw):
                    nc.tensor.matmul(rnb[h][:, off:off + sz],
                                     lhsT=ones_row[:, :],
                                     rhs=rn_bf[:, c0 + off:c0 + off + sz],
                                     start=True, stop=True)
            for jj, (c0, cw) in enumerate(WCH):
                h = 0 if c0 < 2048 else 1
                q0 = c0 - HALves[h][0]
                for d in range(4):
                    nc.vector.tensor_tensor(
                        out=wT8[:, d, c0:c0 + cw],
                        in0=wTf[:, d, c0:c0 + cw],
                        in1=rnb[h][:, q0:q0 + cw], op=AX.mult)

            # ---- deferred t/q work units, interleaved into main loop ----
            def t_unit(k):
                if NO_TPATH:
                    nc.vector.memset(t_sb[:, k:k + 1], 0.0)
                    nc.vector.memset(q_sb[:, k:k + 1], 1.0)
                    return
                e_nat = enat.tile([128, D], F32, name="e_nat")
                nc.sync.dma_start(e_nat[:], emb[k * 128:(k + 1) * 128, :])
                wy = wyp.tile([128, D], F32, name="wy")
                nc.gpsimd.indirect_dma_start(
                    out=wy[:], out_offset=None, in_=wnat[:],
                    in_offset=bass.IndirectOffsetOnAxis(
                        ap=lab_sb[:, k:k + 1], axis=0))
                # NB: vector.tensor_tensor_reduce hangs on this HW — use
                # gpsimd multiply + DVE free-axis reduce instead.
                scr_t = tts.tile([128, D], F32, name="scr_t")
                scr_q = tts.tile([128, D], F32, name="scr_q")
                nc.gpsimd.tensor_mul(scr_t[:], e_nat[:], wy[:])
                nc.vector.reduce_sum(t_sb[:, k:k + 1], scr_t[:],
                                     axis=mybir.AxisListType.X)
                nc.gpsimd.tensor_mul(scr_q[:], wy[:], wy[:])
                nc.vector.reduce_sum(q_sb[:, k:k + 1], scr_q[:],
                                     axis=mybir.AxisListType.X)

            # persistent tail tiles
            t_tot = pers.tile([128, NB], F32)
            q_tot = pers.tile([128, NB], F32)
            ny = pers.tile([128, NB], F32)
            rny = pers.tile([128, NB], F32)
            x = pers.tile([128, NB], F32)
            xsq = pers.tile([128, NB], F32)
            s1mx = pers.tile([128, NB], F32)
            lm = pers.tile([128, NB], F32)
            sb2 = pers.tile([128, NB], F32)
            sub_t = pers.tile([128, NB], F32)
            add_t = pers.tile([128, NB], F32)

            cc1_in = dram.tile([2, 128, NB], F32)
            cc1_out = dram.tile([2, 128, NB], F32)

            def emit_cc1():
                nc.sync.dma_start(cc1_in[0], t_sb[:])
                nc.sync.dma_start(cc1_in[1], q_sb[:])
                nc.gpsimd.collective_compute(
                    "AllReduce", AX.add,
                    replica_groups=[list(range(NCORES))],
                    ins=[cc1_in[:]], outs=[cc1_out[:]])
                nc.sync.dma_start(t_tot[:], cc1_out[0])
                nc.sync.dma_start(q_tot[:], cc1_out[1])

            def emit_margin_chain():
                # cos_y = t/max(||w_y||,eps); x = clip(cos_y)
                nc.scalar.activation(ny[:], q_tot[:], AF.Sqrt)
                nc.vector.tensor_scalar_max(ny[:], ny[:], 1e-12)
                nc.vector.reciprocal(rny[:], ny[:])
                nc.vector.tensor_tensor(out=x[:], in0=t_tot[:], in1=rny[:],
                                        op=AX.mult)
                nc.vector.tensor_scalar(out=x[:], in0=x[:], scalar1=HI,
                                        scalar2=LO_TGT, op0=AX.min,
                                        op1=AX.max)
                nc.vector.tensor_tensor(out=xsq[:], in0=x[:], in1=x[:],
                                        op=AX.mult)
                nc.scalar.activation(s1mx[:], xsq[:], AF.Sqrt, bias=1.0,
                                     scale=-1.0)
                nc.vector.tensor_scalar_mul(lm[:], x[:], SCALE * COS_M)
                nc.vector.tensor_scalar_mul(sb2[:], s1mx[:], SCALE * SIN_M)
                nc.vector.tensor_tensor(out=lm[:], in0=lm[:], in1=sb2[:],
                                        op=AX.subtract)
                nc.scalar.activation(sub_t[:], x[:], AF.Exp,
                                     bias=bias_m64[:], scale=SCALE)
                nc.scalar.activation(add_t[:], lm[:], AF.Exp,
                                     bias=bias_m64[:], scale=1.0)

            # ---- main loop: fp8 DoubleRow matmuls + evac ----
            for nb in range(NB):
                if nb < 16:
                    t_unit(2 * nb)
                    t_unit(2 * nb + 1)
                elif nb == 16:
                    emit_cc1()
                elif nb == 28:
                    emit_margin_chain()
                lhs_lo = nb * 128
                for h, (c0, cw) in enumerate(HALves):
                    ps = pp.tile([128, 2048], F32, name="ps")
                    for off, sz in _mm_chunks(cw):
                        for k in range(2):
                            nc.tensor.matmul(
                                ps[:, off:off + sz],
                                lhsT=eT8[:, 2 * k:2 * k + 2,
                                         lhs_lo:lhs_lo + 128],
                                rhs=wT8[:, 2 * k:2 * k + 2,
                                        c0 + off:c0 + off + sz],
                                start=(k == 0), stop=(k == 1),
                                perf_mode=DR)
                    exp_t = evac.tile([128, 2048], BF16, name="exp_t")
                    nc.scalar.activation(exp_t[:, :cw], ps[:, :cw], AF.Exp,
                                         bias=bias_m64[:],
                                         scale=SCALE / (SE * SW))
                    mn = mns.tile([128, 2048], BF16, name="mn")
                    nc.vector.tensor_scalar(
                        out=mn[:, :cw], in0=exp_t[:, :cw], scalar1=1.0,
                        scalar2=None, op0=AX.min, op1=AX.add,
                        accum_out=scols[:, nb * 2 + h:nb * 2 + h + 1])

            # ---- collective #2: softmax partial sums ----
            s_n = pers.tile([128, NB], F32)
            nc.vector.reduce_sum(
                s_n[:],
                scols[:].rearrange("p (nb h) -> p nb h", h=2),
                axis=mybir.AxisListType.X)
            cc2_in = dram.tile([128, NB], F32)
            cc2_out = dram.tile([128, NB], F32)
            nc.sync.dma_start(cc2_in[:], s_n[:])
            nc.gpsimd.collective_compute(
                "AllReduce", AX.add,
                replica_groups=[list(range(NCORES))],
                ins=[cc2_in[:]], outs=[cc2_out[:]])
            s_tot = pers.tile([128, NB], F32)
            nc.sync.dma_start(s_tot[:], cc2_out[:])

            # ---- tail ----
            sadj = pers.tile([128, NB], F32)
            nc.vector.tensor_tensor(out=sadj[:], in0=s_tot[:], in1=sub_t[:],
                                    op=AX.subtract)
            nc.vector.tensor_tensor(out=sadj[:], in0=sadj[:], in1=add_t[:],
                                    op=AX.add)
            lse = pers.tile([128, NB], F32)
            nc.scalar.activation(lse[:], sadj[:], AF.Ln)
            lossn = pers.tile([128, NB], F32)
            nc.vector.tensor_tensor(out=lossn[:], in0=lse[:], in1=lm[:],
                                    op=AX.subtract)
            red1 = pers.tile([128, 1], F32)
            nc.vector.reduce_sum(red1[:], lossn[:], axis=mybir.AxisListType.X)
            ps_fin = pp.tile([128, 2048], F32, name="ps")
            nc.tensor.matmul(ps_fin[:1, :1], lhsT=ones_col[:], rhs=red1[:],
                             start=True, stop=True)
            out_sb = pers.tile([1, 1], F32)
            nc.scalar.activation(out_sb[:], ps_fin[:1, :1], AF.Copy,
                                 bias=SCALE, scale=1.0 / N)
            nc.sync.dma_start(out[:1, :1], out_sb[:])

    nc.finalize()
    return nc


_NC = None


def _get_nc():
    global _NC
    if _NC is None:
        _NC = _build()
    return _NC


def kernel(embeddings, labels, classifier_weights):
    global LAST_RESULT
    E = np.ascontiguousarray(np.asarray(embeddings, dtype=np.float32))
    W = np.ascontiguousarray(np.asarray(classifier_weights, dtype=np.float32))
    lab = np.asarray(labels).astype(np.int64).ravel()
    assert E.shape == (N, D) and W.shape == (C, D) and lab.shape == (N,)

    nc = _get_nc()
    eT = np.ascontiguousarray(E.T)
    in_maps = []
    for i in range(NCORES):
        lo, hi = i * CREAL, (i + 1) * CREAL
        wT_i = np.zeros((D, CSP), dtype=np.float32)
        wT_i[:, :CREAL] = W[lo:hi].T
        wnat_i = np.zeros((CREAL + 1, D), dtype=np.float32)
        wnat_i[:CREAL] = W[lo:hi]
        loc = np.where((lab >= lo) & (lab < hi), lab - lo, DUMMY)
        lab_i = np.ascontiguousarray(
            loc.reshape(NB, 128).T.astype(np.int32))  # [128, NB]
        in_maps.append({"eT": eT, "emb": E, "wT": wT_i, "wnat": wnat_i,
                        "lab": lab_i})

    res = run_bass_kernel_spmd(nc, in_maps, core_ids=list(range(NCORES)))
    LAST_RESULT = res
    val = np.float32(res.results[0]["out"].reshape(())[()])
    return np.asarray(val, dtype=np.float32).reshape(())
